# revision 1
# baseline (speedup 1.0000x reference)
"""Additive-attention pooling kernel for Trainium2 (8 NeuronCores, data-parallel).

Computes, for inputs x[B,S,D], gru_output[B,S,D], w_omega[D,A], u_omega[A]:
    mask    = sign(|sum_d x|)                       [B,S]
    scores  = tanh(gru @ w_omega) @ u_omega         [B,S]  (masked -> -1e9)
    alphas  = softmax(scores, axis=S)
    out     = einsum('bsd,bs->bd', gru, alphas)     [B,D]

Sharding: batch B=32 split across 8 cores (4 batches/core). Each core runs an
identical Bass program on its shard; params are replicated.

Per-core program (R=8192 rows = 4*2048, processed in 64 chunks of 128 rows):
  - DMA x/gru chunk; DVE row-sum of x -> mask bias (0 or -1e9)
  - PE transposes gru chunk (8x 128x128, via identity) -> PSUM, ACT copies to
    SBUF (rounded to float32r)
  - PE matmul (float32r, full speed): pre_tanh[128,512] accumulated over 8
    k-chunks; ACT tanh
  - DVE fused multiply-by-u + row-reduce -> scores; ACT exp(scores+bias)
  - PE matmul with the softmax numerator as stationary operand accumulates the
    weighted sum AND the denominator in PSUM over the batch's 16 chunks;
    divide once per batch at the end.
"""

import sys

for _p in ("/opt/trn_rl_repo", "/opt/pypackages"):
    if _p not in sys.path:
        sys.path.append(_p)

import numpy as np
import concourse.bacc as bacc
import concourse.tile as tile
from concourse import mybir
from concourse.bass_utils import run_bass_kernel_spmd

F32 = mybir.dt.float32
F32R = mybir.dt.float32r
ACTF = mybir.ActivationFunctionType
ALU = mybir.AluOpType
AX = mybir.AxisListType

B, S, D, A = 32, 2048, 1024, 512
NCORES = 8
BPC = B // NCORES            # batches per core = 4
R = BPC * S                  # rows per core = 8192
NCHUNK = R // 128            # 64 chunks of 128 rows
CPB = S // 128               # chunks per batch = 16
NK = D // 128                # 8 contraction chunks


def build_nc():
    nc = bacc.Bacc("TRN2", target_bir_lowering=False, debug=False)
    x = nc.dram_tensor("x", [R, D], F32, kind="ExternalInput").ap()
    gru = nc.dram_tensor("gru", [R, D], F32R, kind="ExternalInput").ap()
    w = nc.dram_tensor("w", [D, A], F32R, kind="ExternalInput").ap()
    u_b = nc.dram_tensor("u_b", [128, A], F32, kind="ExternalInput").ap()
    ident = nc.dram_tensor("ident", [128, 128], F32R, kind="ExternalInput").ap()
    ones = nc.dram_tensor("ones", [128, 2], F32R, kind="ExternalInput").ap()
    out = nc.dram_tensor("out", [BPC, D], F32, kind="ExternalOutput").ap()

    with tile.TileContext(nc) as tc:
        with (
            tc.tile_pool(name="const", bufs=1) as cpool,
            tc.tile_pool(name="xin", bufs=4) as xpool,
            tc.tile_pool(name="gin", bufs=4) as gpool,
            tc.tile_pool(name="gT", bufs=6) as gTpool,
            tc.tile_pool(name="attn", bufs=3) as apool,
            tc.tile_pool(name="dump", bufs=2) as dpool,
            tc.tile_pool(name="small", bufs=4) as spool,
            tc.tile_pool(name="osb", bufs=4) as opool,
            tc.tile_pool(name="ps_big", bufs=2, space="PSUM") as ps_big_pool,
            tc.tile_pool(name="ps_mm", bufs=1, space="PSUM") as ps_mm_pool,
            tc.tile_pool(name="ps_o", bufs=1, space="PSUM") as ps_o_pool,
        ):
            w_sb = cpool.tile([128, NK, A], F32R, name="w_sb")
            nc.sync.dma_start(w_sb[:], w.rearrange("(k p) a -> p k a", p=128))
            u_sb = cpool.tile([128, A], F32, name="u_sb")
            nc.sync.dma_start(u_sb[:], u_b[:])
            id_sb = cpool.tile([128, 128], F32R, name="id_sb")
            nc.sync.dma_start(id_sb[:], ident[:])
            ones_sb = cpool.tile([128, 2], F32R, name="ones_sb")
            nc.sync.dma_start(ones_sb[:], ones[:])

            for c in range(NCHUNK):
                b = c // CPB
                rows = slice(c * 128, (c + 1) * 128)
                x_t = xpool.tile([128, D], F32, name="x_t", tag="x_t")
                nc.sync.dma_start(x_t[:], x[rows, :])
                g_t = gpool.tile([128, D], F32R, name="g_t", tag="g_t")
                nc.sync.dma_start(g_t[:], gru[rows, :])

                rs = spool.tile([128, 1], F32, name="rs", tag="rs")
                nc.vector.reduce_sum(rs[:], x_t[:], axis=AX.X)
                mb = spool.tile([128, 1], F32, name="mb", tag="mb")
                nc.vector.tensor_scalar(
                    mb[:], rs[:], 0.0, -1e9, op0=ALU.is_equal, op1=ALU.mult
                )

                if c % CPB == 0:
                    ps_o = ps_o_pool.tile([1, 1536], F32, name="ps_o", tag="ps_o")

                ps_big = ps_big_pool.tile(
                    [128, NK * 128], F32R, name="ps_big", tag="ps_big"
                )
                ps_attn = ps_mm_pool.tile([128, A], F32, name="ps_attn", tag="ps_attn")
                for k in range(NK):
                    nc.tensor.transpose(
                        ps_big[:, k * 128 : (k + 1) * 128],
                        g_t[:, k * 128 : (k + 1) * 128],
                        id_sb[:],
                    )
                for k in range(NK):
                    gT = gTpool.tile([128, 128], F32R, name="gT", tag="gT")
                    nc.scalar.copy(gT[:], ps_big[:, k * 128 : (k + 1) * 128])
                    nc.tensor.matmul(
                        ps_attn[:],
                        lhsT=gT[:],
                        rhs=w_sb[:, k, :],
                        start=(k == 0),
                        stop=(k == NK - 1),
                    )

                at = apool.tile([128, A], F32, name="at", tag="at")
                nc.scalar.activation(at[:], ps_attn[:], ACTF.Tanh)

                dump = dpool.tile([128, A], F32, name="dump", tag="dump")
                s_col = spool.tile([128, 1], F32, name="s_col", tag="s_col")
                nc.vector.scalar_tensor_tensor(
                    out=dump[:],
                    in0=at[:],
                    scalar=1.0,
                    in1=u_sb[:],
                    op0=ALU.mult,
                    op1=ALU.mult,
                    accum_out=s_col[:],
                )

                e_col = spool.tile([128, 1], F32R, name="e_col", tag="e_col")
                nc.scalar.activation(
                    e_col[:], s_col[:], ACTF.Exp, bias=mb[:], scale=1.0
                )

                st = (c % CPB) == 0
                sp = (c % CPB) == CPB - 1
                nc.tensor.matmul(
                    ps_o[:, 0:512], lhsT=e_col[:], rhs=g_t[:, 0:512], start=st, stop=sp
                )
                nc.tensor.matmul(
                    ps_o[:, 512:1024],
                    lhsT=e_col[:],
                    rhs=g_t[:, 512:1024],
                    start=st,
                    stop=sp,
                )
                nc.tensor.matmul(
                    ps_o[:, 1024:1026], lhsT=e_col[:], rhs=ones_sb[:], start=st, stop=sp
                )

                if sp:
                    den_sb = opool.tile([1, 1], F32, name="den_sb", tag="den_sb")
                    nc.scalar.copy(den_sb[:], ps_o[:, 1024:1025])
                    rden = opool.tile([1, 1], F32, name="rden", tag="rden")
                    nc.vector.reciprocal(rden[:], den_sb[:])
                    o_sb = opool.tile([1, D], F32, name="o_sb", tag="o_sb")
                    nc.vector.tensor_scalar(
                        o_sb[:], ps_o[:, 0:1024], rden[:], None, op0=ALU.mult
                    )
                    nc.sync.dma_start(out[b : b + 1, :], o_sb[:])
    nc.compile()
    return nc


_NC_CACHE = None


def _get_nc():
    global _NC_CACHE
    if _NC_CACHE is None:
        _NC_CACHE = build_nc()
    return _NC_CACHE


def make_in_maps(x, gru_output, w_omega, u_omega):
    x = np.ascontiguousarray(x, dtype=np.float32)
    gru_output = np.ascontiguousarray(gru_output, dtype=np.float32)
    w_omega = np.ascontiguousarray(w_omega, dtype=np.float32)
    u_b = np.ascontiguousarray(
        np.broadcast_to(u_omega.astype(np.float32), (128, A))
    )
    ident = np.eye(128, dtype=np.float32)
    ones = np.ones((128, 2), dtype=np.float32)
    in_maps = []
    for c in range(NCORES):
        in_maps.append(
            {
                "x": x[c * BPC : (c + 1) * BPC].reshape(R, D),
                "gru": gru_output[c * BPC : (c + 1) * BPC].reshape(R, D),
                "w": w_omega,
                "u_b": u_b,
                "ident": ident,
                "ones": ones,
            }
        )
    return in_maps


def kernel(x, gru_output, w_omega, u_omega):
    nc = _get_nc()
    in_maps = make_in_maps(x, gru_output, w_omega, u_omega)
    res = run_bass_kernel_spmd(nc, in_maps, core_ids=list(range(NCORES)))
    out = np.concatenate(
        [res.results[c]["out"] for c in range(NCORES)], axis=0
    ).astype(np.float32)
    return out


# revision 4
# speedup vs baseline: 7.0072x; 7.0072x over previous
"""Additive-attention pooling kernel for Trainium2 (8 NeuronCores, data-parallel).

Computes, for inputs x[B,S,D], gru_output[B,S,D], w_omega[D,A], u_omega[A]:
    mask    = sign(|sum_d x|)                       [B,S]
    scores  = tanh(gru @ w_omega) @ u_omega         [B,S]  (masked -> -1e9)
    alphas  = softmax(scores, axis=S)
    out     = einsum('bsd,bs->bd', gru, alphas)     [B,D]

Sharding: batch B=32 split across 8 cores (4 batches/core). Each core runs an
identical Bass program on its shard; params are replicated.

Per-core program (R=8192 rows = 4*2048, processed in 64 chunks of 128 rows):
  - DMA x/gru chunk; DVE row-sum of x -> mask bias (0 or -1e9)
  - PE transposes gru chunk (8x 128x128, via identity) -> PSUM, ACT copies to
    SBUF (rounded to float32r)
  - PE matmul (float32r, full speed): pre_tanh[128,512] accumulated over 8
    k-chunks; ACT tanh
  - DVE fused multiply-by-u + row-reduce -> scores; ACT exp(scores+bias)
  - PE matmul with the softmax numerator as stationary operand accumulates the
    weighted sum AND the denominator in PSUM over the batch's 16 chunks;
    divide once per batch at the end.
"""

import contextlib
import sys

for _p in ("/opt/trn_rl_repo", "/opt/pypackages"):
    if _p not in sys.path:
        sys.path.append(_p)

import numpy as np
import concourse.bacc as bacc
import concourse.tile as tile
from concourse import mybir
from concourse.bass_utils import run_bass_kernel_spmd

F32 = mybir.dt.float32
F32R = mybir.dt.float32r
ACTF = mybir.ActivationFunctionType
ALU = mybir.AluOpType
AX = mybir.AxisListType

B, S, D, A = 32, 2048, 1024, 512
NCORES = 8
BPC = B // NCORES            # batches per core = 4
R = BPC * S                  # rows per core = 8192
NCHUNK = R // 128            # 64 chunks of 128 rows
CPB = S // 128               # chunks per batch = 16
NK = D // 128                # 8 contraction chunks


def _emit_body(nc, tc, t):
    """Emit the full per-core computation. `t` holds the SBUF constant tiles
    and DRAM APs."""
    for c in range(NCHUNK):
        b = c // CPB
        rows = slice(c * 128, (c + 1) * 128)
        x_t = t["xpool"].tile([128, D], F32, name="x_t", tag="x_t")
        nc.sync.dma_start(x_t[:], t["x"][rows, :])
        g_t = t["gpool"].tile([128, D], F32R, name="g_t", tag="g_t")
        nc.sync.dma_start(g_t[:], t["gru"][rows, :])

        rs = t["spool"].tile([128, 1], F32, name="rs", tag="rs")
        nc.vector.reduce_sum(rs[:], x_t[:], axis=AX.X)
        mb = t["spool"].tile([128, 1], F32, name="mb", tag="mb")
        nc.vector.tensor_scalar(
            mb[:], rs[:], 0.0, -1e9, op0=ALU.is_equal, op1=ALU.mult
        )

        if c % CPB == 0:
            t["ps_o"] = t["ps_o_pool"].tile([1, 1536], F32, name="ps_o", tag="ps_o")
        ps_o = t["ps_o"]

        ps_big = t["ps_big_pool"].tile(
            [128, NK * 128], F32R, name="ps_big", tag="ps_big"
        )
        ps_attn = t["ps_mm_pool"].tile([128, A], F32, name="ps_attn", tag="ps_attn")
        for k in range(NK):
            nc.tensor.transpose(
                ps_big[:, k * 128 : (k + 1) * 128],
                g_t[:, k * 128 : (k + 1) * 128],
                t["id_sb"][:],
            )
        for k in range(NK):
            gT = t["gTpool"].tile([128, 128], F32R, name="gT", tag="gT")
            nc.scalar.copy(gT[:], ps_big[:, k * 128 : (k + 1) * 128])
            nc.tensor.matmul(
                ps_attn[:],
                lhsT=gT[:],
                rhs=t["w_sb"][:, k, :],
                start=(k == 0),
                stop=(k == NK - 1),
            )

        at = t["apool"].tile([128, A], F32, name="at", tag="at")
        nc.scalar.activation(at[:], ps_attn[:], ACTF.Tanh)

        dump = t["dpool"].tile([128, A], F32, name="dump", tag="dump")
        s_col = t["spool"].tile([128, 1], F32, name="s_col", tag="s_col")
        nc.vector.scalar_tensor_tensor(
            out=dump[:],
            in0=at[:],
            scalar=1.0,
            in1=t["u_sb"][:],
            op0=ALU.mult,
            op1=ALU.mult,
            accum_out=s_col[:],
        )

        e_col = t["spool"].tile([128, 1], F32R, name="e_col", tag="e_col")
        nc.scalar.activation(e_col[:], s_col[:], ACTF.Exp, bias=mb[:], scale=1.0)

        st = (c % CPB) == 0
        sp = (c % CPB) == CPB - 1
        nc.tensor.matmul(
            ps_o[:, 0:512], lhsT=e_col[:], rhs=g_t[:, 0:512], start=st, stop=sp
        )
        nc.tensor.matmul(
            ps_o[:, 512:1024], lhsT=e_col[:], rhs=g_t[:, 512:1024], start=st, stop=sp
        )
        nc.tensor.matmul(
            ps_o[:, 1024:1026], lhsT=e_col[:], rhs=t["ones_sb"][:], start=st, stop=sp
        )

        if sp:
            den_sb = t["opool"].tile([1, 1], F32, name="den_sb", tag="den_sb")
            nc.scalar.copy(den_sb[:], ps_o[:, 1024:1025])
            rden = t["opool"].tile([1, 1], F32, name="rden", tag="rden")
            nc.vector.reciprocal(rden[:], den_sb[:])
            o_sb = t["opool"].tile([1, D], F32, name="o_sb", tag="o_sb")
            nc.vector.tensor_scalar(
                o_sb[:], ps_o[:, 0:1024], rden[:], None, op0=ALU.mult
            )
            nc.sync.dma_start(t["out"][b : b + 1, :], o_sb[:])


def build_nc(loop_iters=None):
    """loop_iters: when set, wraps the whole per-core body in a device-side
    For_i repeating it N times — used only by the timing harness."""
    nc = bacc.Bacc("TRN2", target_bir_lowering=False, debug=False)
    t = {}
    t["x"] = nc.dram_tensor("x", [R, D], F32, kind="ExternalInput").ap()
    t["gru"] = nc.dram_tensor("gru", [R, D], F32R, kind="ExternalInput").ap()
    t["w"] = nc.dram_tensor("w", [D, A], F32R, kind="ExternalInput").ap()
    t["u_b"] = nc.dram_tensor("u_b", [128, A], F32, kind="ExternalInput").ap()
    t["ident"] = nc.dram_tensor("ident", [128, 128], F32R, kind="ExternalInput").ap()
    t["ones"] = nc.dram_tensor("ones", [128, 2], F32R, kind="ExternalInput").ap()
    t["out"] = nc.dram_tensor("out", [BPC, D], F32, kind="ExternalOutput").ap()

    with tile.TileContext(nc) as tc:
        with (
            tc.tile_pool(name="const", bufs=1) as cpool,
            tc.tile_pool(name="xin", bufs=4) as xpool,
            tc.tile_pool(name="gin", bufs=4) as gpool,
            tc.tile_pool(name="gT", bufs=6) as gTpool,
            tc.tile_pool(name="attn", bufs=3) as apool,
            tc.tile_pool(name="dump", bufs=2) as dpool,
            tc.tile_pool(name="small", bufs=4) as spool,
            tc.tile_pool(name="osb", bufs=4) as opool,
            tc.tile_pool(name="ps_big", bufs=2, space="PSUM") as ps_big_pool,
            tc.tile_pool(name="ps_mm", bufs=1, space="PSUM") as ps_mm_pool,
            tc.tile_pool(name="ps_o", bufs=1, space="PSUM") as ps_o_pool,
        ):
            t.update(
                xpool=xpool, gpool=gpool, gTpool=gTpool, apool=apool,
                dpool=dpool, spool=spool, opool=opool,
                ps_big_pool=ps_big_pool, ps_mm_pool=ps_mm_pool,
                ps_o_pool=ps_o_pool,
            )
            w_sb = cpool.tile([128, NK, A], F32R, name="w_sb")
            nc.sync.dma_start(w_sb[:], t["w"].rearrange("(k p) a -> p k a", p=128))
            u_sb = cpool.tile([128, A], F32, name="u_sb")
            nc.sync.dma_start(u_sb[:], t["u_b"][:])
            id_sb = cpool.tile([128, 128], F32R, name="id_sb")
            nc.sync.dma_start(id_sb[:], t["ident"][:])
            ones_sb = cpool.tile([128, 2], F32R, name="ones_sb")
            nc.sync.dma_start(ones_sb[:], t["ones"][:])
            t.update(w_sb=w_sb, u_sb=u_sb, id_sb=id_sb, ones_sb=ones_sb)

            loop_cm = (
                tc.For_i(0, loop_iters, 1)
                if loop_iters is not None
                else contextlib.nullcontext()
            )
            with loop_cm:
                _emit_body(nc, tc, t)
    nc.compile()
    return nc


_NC_CACHE = None


def _get_nc():
    global _NC_CACHE
    if _NC_CACHE is None:
        _NC_CACHE = build_nc()
    return _NC_CACHE


def make_in_maps(x, gru_output, w_omega, u_omega):
    x = np.ascontiguousarray(x, dtype=np.float32)
    gru_output = np.ascontiguousarray(gru_output, dtype=np.float32)
    w_omega = np.ascontiguousarray(w_omega, dtype=np.float32)
    u_b = np.ascontiguousarray(
        np.broadcast_to(u_omega.astype(np.float32), (128, A))
    )
    ident = np.eye(128, dtype=np.float32)
    ones = np.ones((128, 2), dtype=np.float32)
    in_maps = []
    for c in range(NCORES):
        in_maps.append(
            {
                "x": x[c * BPC : (c + 1) * BPC].reshape(R, D),
                "gru": gru_output[c * BPC : (c + 1) * BPC].reshape(R, D),
                "w": w_omega,
                "u_b": u_b,
                "ident": ident,
                "ones": ones,
            }
        )
    return in_maps


def kernel(x, gru_output, w_omega, u_omega):
    nc = _get_nc()
    in_maps = make_in_maps(x, gru_output, w_omega, u_omega)
    res = run_bass_kernel_spmd(nc, in_maps, core_ids=list(range(NCORES)))
    out = np.concatenate(
        [res.results[c]["out"] for c in range(NCORES)], axis=0
    ).astype(np.float32)
    return out
